# revision 29
# baseline (speedup 1.0000x reference)
"""Trainium2 Bass kernel for nn_Conv1DTokenEncoder.

Math: per (b,t) row of length L=1024,
  out[r, :] = [total, x0, x1, xL2, xL1, 1, 0, 0] @ M8
with M8 rows [wsum/L, -(w3+w4)/L, -w4/L, -w0/L, -(w0+w1)/L, bias, 0, 0]
(host-precomputed), i.e. the K=5 same-pad conv + mean-over-L collapses to
a row total, 4 edge elements, and a tiny K=8 matmul.

The 2e-2 rel-err gate leaves a huge precision budget, so the kernel runs
a low-precision memory format end to end (measured scale-rel err ~5e-3):
- input x is host-cast to fp8 E3M4 (4 mantissa bits) -> 4 MiB HBM read
  per core instead of 16 MiB;
- output is written fp16 -> 4 MiB per core instead of 8 MiB, upcast to
  f32 on host;
- features / M8 / PE matmuls in fp16 (e10m11 multiply, f32 accumulate),
  row-total accumulation in f32 on-engine.

Device structure (per core, 4096 rows; P=128 partitions x G=8 rows each,
4 blocks of 1024 rows):
- input DMA per block in two 512 KiB halves on the sync HWDGE ring
  (8 KiB contiguous DRAM runs per partition);
- row totals split across engines (both run 1 elem/cycle/partition on
  TRN2 hw): 4 groups via ScalarE activation-accumulate + 4 groups in one
  strided DVE tensor_reduce, accumulating f32;
- VectorE edge-column copies (fp8->fp16 cast), PE transposes of the
  [128, 4*32] fp16 feature tile, one [128,128] PSUM->SBUF fp16 copy per
  transpose, G matmuls [8,128]x[8,512] fp16 -> f32 PSUM (M8 replicated at
  partition bases 0/32/64/96, explicit tile_position);
- PSUM->SBUF output copies (f32 -> fp16 cast, 2 groups per instruction)
  alternating Vector/Scalar; one batched [128, G, 512] fp16 output DMA
  per block on gpsimd SWDGE.
Pure data parallel across 8 cores (batch*token rows sharded).
Measured: ~42-44 us/iteration on hw (baseline f32 version: ~95-103 us);
scale-relative error 4.6e-3 vs the 2e-2 gate.
"""

import numpy as np
import ml_dtypes

B, T, L, D = 16, 2048, 1024, 512
N_CORES = 8
BT = B * T
ROWS_PER_CORE = BT // N_CORES  # 4096
P = 128

G = 8                       # rows per partition per block
BLOCK_ROWS = P * G          # 1024
N_BLOCKS = ROWS_PER_CORE // BLOCK_ROWS  # 4
ACT_GROUPS = 4              # row-total groups reduced on ScalarE (rest on DVE)
NF = 8                      # feature count (padded)

_CACHE = {}


def _build(repeat: int = 1):
    import concourse.bass as bass
    import concourse.tile as tile
    from concourse import bacc, mybir

    f32 = mybir.dt.float32
    f16 = mybir.dt.float16
    f8 = mybir.dt.float8e3
    nc = bacc.Bacc("TRN2", target_bir_lowering=False, debug=False)

    x_d = nc.dram_tensor("x", [ROWS_PER_CORE, L], f8, kind="ExternalInput")
    m_d = nc.dram_tensor("m8", [4, NF, D], f16, kind="ExternalInput")
    id_d = nc.dram_tensor("ident", [P, P], f16, kind="ExternalInput")
    o_d = nc.dram_tensor("out", [ROWS_PER_CORE, D], f16, kind="ExternalOutput")

    AF = mybir.ActivationFunctionType
    AX = mybir.AxisListType
    ALU = mybir.AluOpType
    x_v = x_d.ap().rearrange("(nb p g) l -> nb p g l", p=P, g=G)
    o_v = o_d.ap().rearrange("(nb p g) d -> nb p g d", p=P, g=G)
    n_trans = (G + 3) // 4

    with tile.TileContext(nc) as tc:
        with (
            tc.tile_pool(name="const", bufs=1) as constp,
            tc.tile_pool(name="xin", bufs=3) as xin,
            tc.tile_pool(name="actout", bufs=2) as actoutp,
            tc.tile_pool(name="fred", bufs=2) as fredp,
            tc.tile_pool(name="feat", bufs=2) as featp,
            tc.tile_pool(name="ftT_ps", bufs=2, space="PSUM") as ftp,
            tc.tile_pool(name="ftT_sb", bufs=2) as fts,
            tc.tile_pool(name="out_ps", bufs=3, space="PSUM") as outp,
            tc.tile_pool(name="out_sb", bufs=2) as outs,
        ):
            # M8 replicated at partition bases 0/32/64/96 so each matmul's
            # lhsT (a 32-aligned slice of the transposed feature tile) and
            # rhs share a base partition
            m8 = constp.tile([P, D], f16)
            for t in range(4):
                nc.sync.dma_start(m8[32 * t : 32 * t + NF, :], m_d[t])
            ident = constp.tile([P, P], f16)
            nc.sync.dma_start(ident[:], id_d[:])

            def body():
                for i in range(N_BLOCKS):
                    xt = xin.tile([P, G, L], f8)
                    ft = featp.tile([P, G, 32], f16)
                    fred = fredp.tile([P, G], f32)
                    ot = outs.tile([P, G, D], f16)
                    h = G // 2
                    if i == 0:
                        # quarter-granularity input for the first block so
                        # the ScalarE reduces restart ~0.8us sooner after
                        # the loop-boundary barrier
                        for qq in range(4):
                            nc.sync.dma_start(
                                xt[:, 2 * qq : 2 * qq + 2, :],
                                x_v[i, :, 2 * qq : 2 * qq + 2, :],
                            )
                    else:
                        nc.sync.dma_start(xt[:, :h, :], x_v[i, :, :h, :])
                        nc.sync.dma_start(xt[:, h:, :], x_v[i, :, h:, :])

                    # row totals, split across engines (both 1 elem/cycle on
                    # TRN2 hw): first half's groups on ScalarE activation-
                    # accumulate, second half's on DVE tensor_scalar.
                    # Emitted before any downstream-dependent op so neither
                    # engine's in-order queue head-of-line blocks on PE.
                    for j in range(h):
                        sc = actoutp.tile([P, L], f8)
                        nc.scalar.activation(
                            sc[:], xt[:, j, :], AF.Copy,
                            accum_out=fred[:, j : j + 1],
                        )
                    # remaining row totals in one strided DVE reduce
                    nc.vector.tensor_reduce(
                        fred[:, h:], xt[:, h:, :], axis=AX.X, op=ALU.add,
                    )

                    # feature tile: col0=total, 1:3=x0,x1, 3:5=xL2,xL1,
                    # 5=ones, 6:8=zeros, 8:32 pad (must stay finite: flows
                    # through transpose+copy)
                    nc.vector.tensor_copy(ft[:, :, 1:3], xt[:, :, 0:2])
                    nc.vector.tensor_copy(ft[:, :, 3:5], xt[:, :, L - 2 : L])
                    nc.vector.memset(ft[:, :, 5:6], 1.0)
                    nc.vector.memset(ft[:, :, 6:32], 0.0)
                    nc.vector.tensor_copy(
                        ft[:, :, 0:1],
                        fred[:].rearrange("p (g o) -> p g o", o=1),
                    )

                    # transpose features per half; 32-col groups keep
                    # transposed slices at 32-aligned partition bases
                    ftTs = []
                    for t in range(n_trans):
                        ftT_p = ftp.tile([P, P], f16)
                        nc.tensor.transpose(
                            ftT_p[:],
                            ft[:, 4 * t : 4 * t + 4, :].rearrange(
                                "p g c -> p (g c)"
                            ),
                            ident[:],
                        )
                        ftT = fts.tile([P, P], f16)
                        nc.vector.tensor_copy(ftT[:], ftT_p[:])
                        ftTs.append(ftT)

                    for jp in range(G // 2):
                        # two matmuls share a [128, 2, 512] PSUM tile so
                        # each PSUM->SBUF fp16 copy moves 2 groups
                        op = outp.tile([P, 2, D], f32)
                        for q in range(2):
                            j = 2 * jp + q
                            jj = j % 4
                            nc.tensor.matmul(
                                op[:, q, :],
                                ftTs[j // 4][32 * jj : 32 * jj + NF, :],
                                m8[32 * jj : 32 * jj + NF, :],
                                tile_position=(32 * jj, 0),
                            )
                        # copies alternate Vector/Scalar
                        if jp % 2 == 0:
                            nc.vector.tensor_copy(
                                ot[:, 2 * jp : 2 * jp + 2, :], op[:]
                            )
                        else:
                            nc.scalar.activation(
                                ot[:, 2 * jp : 2 * jp + 2, :], op[:], AF.Copy
                            )
                            # half-block output DMA on the gpsimd SWDGE
                            # ring as soon as its 4 groups are copied, so
                            # the tail drains earlier
                            g0 = 2 * (jp - 1)
                            nc.gpsimd.dma_start(
                                o_v[i, :, g0 : g0 + 4, :],
                                ot[:, g0 : g0 + 4, :],
                            )

            if repeat == 1:
                body()
            else:
                with tc.For_i(0, repeat, 1):
                    body()

    nc.compile()
    return nc


def _host_m8(w: np.ndarray, b: np.ndarray) -> np.ndarray:
    w = w.astype(np.float64)
    invL = 1.0 / L
    rows = [
        w.sum(axis=1) * invL,            # total
        -(w[:, 3] + w[:, 4]) * invL,     # x[0]
        -w[:, 4] * invL,                 # x[1]
        -w[:, 0] * invL,                 # x[L-2]
        -(w[:, 0] + w[:, 1]) * invL,     # x[L-1]
        b.astype(np.float64),            # ones
        np.zeros(D, np.float64),
        np.zeros(D, np.float64),
    ]
    m8 = np.stack(rows).astype(np.float16)
    return np.tile(m8[None], (4, 1, 1))


def make_in_maps(inputs: dict) -> list:
    m8 = _host_m8(np.asarray(inputs["w"]), np.asarray(inputs["b"]))
    ident = np.eye(P, dtype=np.float16)
    x8 = (
        np.ascontiguousarray(np.asarray(inputs["x"], dtype=np.float32))
        .reshape(BT, L)
        .astype(ml_dtypes.float8_e3m4)
        .reshape(N_CORES, ROWS_PER_CORE, L)
    )
    return [{"x": x8[i], "m8": m8, "ident": ident} for i in range(N_CORES)]


def kernel(x: np.ndarray, w: np.ndarray, b: np.ndarray) -> np.ndarray:
    from concourse.bass_utils import run_bass_kernel_spmd

    if "nc" not in _CACHE:
        _CACHE["nc"] = _build()
    nc = _CACHE["nc"]

    in_maps = make_in_maps({"x": x, "w": w, "b": b})
    res = run_bass_kernel_spmd(nc, in_maps, list(range(N_CORES))).results
    out = np.concatenate(
        [res[i]["out"].astype(np.float32) for i in range(N_CORES)], axis=0
    )
    return out.reshape(B, T, D)


# revision 31
# speedup vs baseline: 1.0907x; 1.0907x over previous
"""Trainium2 Bass kernel for nn_Conv1DTokenEncoder.

Math: per (b,t) row of length L=1024,
  out[r, :] = [total, x0, x1, xL2, xL1, 1, 0, 0] @ M8
with M8 rows [wsum/L, -(w3+w4)/L, -w4/L, -w0/L, -(w0+w1)/L, bias, 0, 0]
(host-precomputed), i.e. the K=5 same-pad conv + mean-over-L collapses to
a row total, 4 edge elements, and a tiny K=8 matmul.

The 2e-2 rel-err gate leaves a huge precision budget, so the kernel runs
a low-precision memory format end to end (measured scale-rel err ~5e-3):
- input x is host-cast to fp8 E3M4 (4 mantissa bits) -> 4 MiB HBM read
  per core instead of 16 MiB;
- output is written fp16 -> 4 MiB per core instead of 8 MiB, upcast to
  f32 on host;
- features / M8 / PE matmuls in fp16 (e10m11 multiply, f32 accumulate),
  row-total accumulation in f32 on-engine.

Device structure (per core, 4096 rows; P=128 partitions x G=8 rows each,
4 blocks of 1024 rows):
- input DMA per block in two 512 KiB halves on the sync HWDGE ring
  (8 KiB contiguous DRAM runs per partition);
- row totals split across engines (both run 1 elem/cycle/partition on
  TRN2 hw): 4 groups via ScalarE activation-accumulate + 4 groups in one
  strided DVE tensor_reduce, accumulating f32;
- VectorE edge-column copies (fp8->fp16 cast), PE transposes of the
  [128, 4*32] fp16 feature tile, one [128,128] PSUM->SBUF fp16 copy per
  transpose, G matmuls [8,128]x[8,512] fp16 -> f32 PSUM (M8 replicated at
  partition bases 0/32/64/96, explicit tile_position);
- PSUM->SBUF output copies (f32 -> fp16 cast, 2 groups per instruction)
  alternating Vector/Scalar; one batched [128, G, 512] fp16 output DMA
  per block on gpsimd SWDGE.
Pure data parallel across 8 cores (batch*token rows sharded).
Measured: ~42-44 us/iteration on hw (baseline f32 version: ~95-103 us);
scale-relative error 4.6e-3 vs the 2e-2 gate.
"""

import numpy as np
import ml_dtypes

B, T, L, D = 16, 2048, 1024, 512
N_CORES = 8
BT = B * T
ROWS_PER_CORE = BT // N_CORES  # 4096
P = 128

G = 8                       # rows per partition per block
BLOCK_ROWS = P * G          # 1024
N_BLOCKS = ROWS_PER_CORE // BLOCK_ROWS  # 4
ACT_GROUPS = 4              # row-total groups reduced on ScalarE (rest on DVE)
NF = 8                      # feature count (padded)

_CACHE = {}


def _build(repeat: int = 1):
    import concourse.bass as bass
    import concourse.tile as tile
    from concourse import bacc, mybir

    f32 = mybir.dt.float32
    f16 = mybir.dt.float16
    f8 = mybir.dt.float8e3
    nc = bacc.Bacc("TRN2", target_bir_lowering=False, debug=False)

    x_d = nc.dram_tensor("x", [ROWS_PER_CORE, L], f8, kind="ExternalInput")
    m_d = nc.dram_tensor("m8", [4, NF, D], f16, kind="ExternalInput")
    id_d = nc.dram_tensor("ident", [P, P], f16, kind="ExternalInput")
    o_d = nc.dram_tensor("out", [ROWS_PER_CORE, D], f16, kind="ExternalOutput")

    AF = mybir.ActivationFunctionType
    AX = mybir.AxisListType
    ALU = mybir.AluOpType
    x_v = x_d.ap().rearrange("(nb p g) l -> nb p g l", p=P, g=G)
    o_v = o_d.ap().rearrange("(nb p g) d -> nb p g d", p=P, g=G)
    n_trans = (G + 3) // 4

    with tile.TileContext(nc) as tc:
        with (
            tc.tile_pool(name="const", bufs=1) as constp,
            tc.tile_pool(name="xin", bufs=3) as xin,
            tc.tile_pool(name="actout", bufs=2) as actoutp,
            tc.tile_pool(name="fred", bufs=2) as fredp,
            tc.tile_pool(name="feat", bufs=2) as featp,
            tc.tile_pool(name="ftT_ps", bufs=2, space="PSUM") as ftp,
            tc.tile_pool(name="ftT_sb", bufs=2) as fts,
            tc.tile_pool(name="out_ps", bufs=3, space="PSUM") as outp,
            tc.tile_pool(name="out_sb", bufs=2) as outs,
        ):
            # M8 replicated at partition bases 0/32/64/96 so each matmul's
            # lhsT (a 32-aligned slice of the transposed feature tile) and
            # rhs share a base partition. Const loads go on the idle gpsimd
            # ring so they don't delay block 0's input DMAs on the sync
            # ring in a single-shot dispatch.
            m8 = constp.tile([P, D], f16)
            for t in range(4):
                nc.gpsimd.dma_start(m8[32 * t : 32 * t + NF, :], m_d[t])
            ident = constp.tile([P, P], f16)
            nc.gpsimd.dma_start(ident[:], id_d[:])

            def body():
                for i in range(N_BLOCKS):
                    xt = xin.tile([P, G, L], f8)
                    ft = featp.tile([P, G, 32], f16)
                    fred = fredp.tile([P, G], f32)
                    ot = outs.tile([P, G, D], f16)
                    h = G // 2
                    # quarter-granularity input, interleaved between the
                    # two reduce engines' group sets ({0,1},{4,5},{2,3},
                    # {6,7}) so ScalarE AND VectorE both have work after
                    # the first transfers instead of DVE idling until the
                    # second half lands
                    for g0 in (0, 4, 2, 6):
                        nc.sync.dma_start(
                            xt[:, g0 : g0 + 2, :],
                            x_v[i, :, g0 : g0 + 2, :],
                        )

                    # row totals, split across engines (both 1 elem/cycle
                    # on TRN2 hw): groups 0-3 on ScalarE activation-
                    # accumulate, groups 4-7 in two strided DVE reduces.
                    # Emitted before any downstream-dependent op so neither
                    # engine's in-order queue head-of-line blocks on PE.
                    for j in range(h):
                        sc = actoutp.tile([P, L], f8)
                        nc.scalar.activation(
                            sc[:], xt[:, j, :], AF.Copy,
                            accum_out=fred[:, j : j + 1],
                        )
                    nc.vector.tensor_reduce(
                        fred[:, 4:6], xt[:, 4:6, :], axis=AX.X, op=ALU.add,
                    )
                    nc.vector.tensor_reduce(
                        fred[:, 6:8], xt[:, 6:8, :], axis=AX.X, op=ALU.add,
                    )

                    # feature tile: col0=total, 1:3=x0,x1, 3:5=xL2,xL1,
                    # 5=ones, 6:8=zeros, 8:32 pad (must stay finite: flows
                    # through transpose+copy)
                    nc.vector.tensor_copy(ft[:, :, 1:3], xt[:, :, 0:2])
                    nc.vector.tensor_copy(ft[:, :, 3:5], xt[:, :, L - 2 : L])
                    nc.vector.memset(ft[:, :, 5:6], 1.0)
                    nc.vector.memset(ft[:, :, 6:32], 0.0)
                    nc.vector.tensor_copy(
                        ft[:, :, 0:1],
                        fred[:].rearrange("p (g o) -> p g o", o=1),
                    )

                    # transpose features per half; 32-col groups keep
                    # transposed slices at 32-aligned partition bases
                    ftTs = []
                    for t in range(n_trans):
                        ftT_p = ftp.tile([P, P], f16)
                        nc.tensor.transpose(
                            ftT_p[:],
                            ft[:, 4 * t : 4 * t + 4, :].rearrange(
                                "p g c -> p (g c)"
                            ),
                            ident[:],
                        )
                        ftT = fts.tile([P, P], f16)
                        nc.vector.tensor_copy(ftT[:], ftT_p[:])
                        ftTs.append(ftT)

                    for jp in range(G // 2):
                        # two matmuls share a [128, 2, 512] PSUM tile so
                        # each PSUM->SBUF fp16 copy moves 2 groups
                        op = outp.tile([P, 2, D], f32)
                        for q in range(2):
                            j = 2 * jp + q
                            jj = j % 4
                            nc.tensor.matmul(
                                op[:, q, :],
                                ftTs[j // 4][32 * jj : 32 * jj + NF, :],
                                m8[32 * jj : 32 * jj + NF, :],
                                tile_position=(32 * jj, 0),
                            )
                        # copies alternate Vector/Scalar
                        if jp % 2 == 0:
                            nc.vector.tensor_copy(
                                ot[:, 2 * jp : 2 * jp + 2, :], op[:]
                            )
                        else:
                            nc.scalar.activation(
                                ot[:, 2 * jp : 2 * jp + 2, :], op[:], AF.Copy
                            )
                            # half-block output DMA on the gpsimd SWDGE
                            # ring as soon as its 4 groups are copied, so
                            # the tail drains earlier
                            g0 = 2 * (jp - 1)
                            nc.gpsimd.dma_start(
                                o_v[i, :, g0 : g0 + 4, :],
                                ot[:, g0 : g0 + 4, :],
                            )

            if repeat == 1:
                body()
            else:
                with tc.For_i(0, repeat, 1):
                    body()

    nc.compile()
    return nc


def _host_m8(w: np.ndarray, b: np.ndarray) -> np.ndarray:
    w = w.astype(np.float64)
    invL = 1.0 / L
    rows = [
        w.sum(axis=1) * invL,            # total
        -(w[:, 3] + w[:, 4]) * invL,     # x[0]
        -w[:, 4] * invL,                 # x[1]
        -w[:, 0] * invL,                 # x[L-2]
        -(w[:, 0] + w[:, 1]) * invL,     # x[L-1]
        b.astype(np.float64),            # ones
        np.zeros(D, np.float64),
        np.zeros(D, np.float64),
    ]
    m8 = np.stack(rows).astype(np.float16)
    return np.tile(m8[None], (4, 1, 1))


def make_in_maps(inputs: dict) -> list:
    m8 = _host_m8(np.asarray(inputs["w"]), np.asarray(inputs["b"]))
    ident = np.eye(P, dtype=np.float16)
    x8 = (
        np.ascontiguousarray(np.asarray(inputs["x"], dtype=np.float32))
        .reshape(BT, L)
        .astype(ml_dtypes.float8_e3m4)
        .reshape(N_CORES, ROWS_PER_CORE, L)
    )
    return [{"x": x8[i], "m8": m8, "ident": ident} for i in range(N_CORES)]


def kernel(x: np.ndarray, w: np.ndarray, b: np.ndarray) -> np.ndarray:
    from concourse.bass_utils import run_bass_kernel_spmd

    if "nc" not in _CACHE:
        _CACHE["nc"] = _build()
    nc = _CACHE["nc"]

    in_maps = make_in_maps({"x": x, "w": w, "b": b})
    res = run_bass_kernel_spmd(nc, in_maps, list(range(N_CORES))).results
    out = np.concatenate(
        [res[i]["out"].astype(np.float32) for i in range(N_CORES)], axis=0
    )
    return out.reshape(B, T, D)
